# revision 87
# baseline (speedup 1.0000x reference)
"""FAGCN (FAConv x3) Trainium2 kernel, 8-core SPMD — v2.

Sharding: nodes partitioned across 8 cores (6250 each, padded to 6272).
Edges assigned to the owner of dst. Node state is kept FEATURE-MAJOR
([128 feat, nodes]) so phase A/C need no transposes and per-window
al/ar are PE matvecs.

Per layer: each core computes its table slice (rows = [h*dinv (128
bf16) | al | pad] = 512B), AllGathers the table, then runs an edge
pass over 128-edge chunks grouped by 64-node dst window (64 halves
the tanh/one-hot/segsum widths on the bottleneck Act engine; the
~14% extra chunk padding costs less than it saves):
  - dma_gather of rows by src (two int16 halves; <=1024 descriptors
    per call -- the SWDGE ring hangs the device beyond that),
  - per segment (<=4 chunks, one dst window): rank-1 PE matmul
    broadcasts ar_w into PSUM; per chunk one Act tanh with the
    per-edge al bias read straight from the gathered row -> tt,
  - per chunk: one-hot tensor_scalar (is_equal, 4x bf16 mode); per
    segment a wide tensor_tensor multiply by tt (2x bf16 mode) and
    per chunk a TensorE segment-sum matmul accumulating [feat, dst]
    in PSUM (lhsT=gathered rows, rhs=one-hot),
  - group end: DVE adds PSUM into the wide accumulator.
h_new = dinv * acc + EPS*h0 as two wide DVE ops per layer.
Phase C batches all Exp then one wide Ln (activation-table loads are
1.3us each; Relu/Tanh/Copy/Exp share one table set, Ln does not).
"""
import ml_dtypes
import numpy as np

import concourse.bacc as bacc
import concourse.bass as bass
import concourse.mybir as mybir
import concourse.tile as tile
from concourse.bass_utils import run_bass_kernel_spmd
from concourse.masks import make_identity

F32 = mybir.dt.float32
BF16 = mybir.dt.bfloat16
I16 = mybir.dt.int16

EPS = 0.1


class Cfg:
    def __init__(self, n_nodes, n_edges, in_dim, out_dim, n_layers,
                 n_cores=8, split=32768, csup=8, segn=4, bf16=True):
        self.BF16 = bf16
        self.N = n_nodes
        self.E = n_edges
        self.IN = in_dim
        self.H = 128
        self.OUT = out_dim
        self.NL = n_layers
        self.NC = n_cores
        self.NV = n_nodes // n_cores          # owned nodes per core
        assert self.NV * n_cores == n_nodes
        self.W = (self.NV + 127) // 128       # dst windows per core
        self.NP = self.W * 128                # padded nodes per core
        self.NPG = self.NP * n_cores          # padded global nodes
        self.KT = in_dim // 128               # k-tiles of the input matmul
        assert in_dim % 128 == 0
        self.SPLIT = split                    # int16 table-half boundary
        self.CSUP = csup                      # chunks per gather call
        self.DW = 64                          # dst-window width (nodes)
        self.NW = self.NP // self.DW          # dst windows per core
        self.SEGN = segn                      # chunks per tanh segment
        self.AB = 2                           # node-tiles per x load
        self.GPB = 6                          # gather pool bufs
        self.OHB = 6                          # one-hot / tt pool bufs
        self.TTMODE = "bias"                  # "rank" | "bias" tanh build
        # SWDGE descriptor ring (carved from SBUF): csup*128 descs * 16B.
        # >1024 descs per gather hangs the device at the default 16 KiB.
        self.DDSS = max(16384, csup * 128 * 16)


FULL = Cfg(50000, 600000, 512, 64, 3)


# ----------------------------------------------------------------- planner

def plan_edges(cfg, edge_index):
    """Host-side edge sharding. Returns the uniform chunk schedule and the
    per-core packed arrays."""
    src = edge_index[0].astype(np.int64)
    dst = edge_index[1].astype(np.int64)
    owner = dst // cfg.NV
    remap = (src // cfg.NV) * cfg.NP + (src % cfg.NV)   # padded global row id

    per_core = []
    counts = np.zeros((cfg.NC, 2, cfg.NW), np.int64)
    for c in range(cfg.NC):
        m = owner == c
        s_r = remap[m]
        d_l = dst[m] - c * cfg.NV
        w = d_l // cfg.DW
        h = (s_r >= cfg.SPLIT).astype(np.int64)
        order = np.lexsort((d_l, w, h))
        s_r, d_l, w, h = s_r[order], d_l[order], w[order], h[order]
        for hh in range(2):
            for ww in range(cfg.NW):
                counts[c, hh, ww] = np.count_nonzero((h == hh) & (w == ww))
        per_core.append((s_r, d_l, h, w))

    nch = np.maximum((counts.max(axis=0) + 127) // 128, 0)  # [2, W] chunks
    nch[counts.max(axis=0) == 0] = 0
    NCH = int(nch.sum())
    EPAD = NCH * 128

    # global chunk meta: (half, window, first_of_group, last_of_group)
    chunk_meta = []
    for hh in range(2):
        for ww in range(cfg.NW):
            n = int(nch[hh, ww])
            for k in range(n):
                chunk_meta.append((hh, ww, k == 0, k == n - 1))
    nch0 = int(nch[0].sum())   # chunks in half 0

    cores = []
    for c in range(cfg.NC):
        s_r, d_l, h, w = per_core[c]
        gidx = np.zeros(EPAD, np.int64)     # table row (half-rebased)
        rel = np.full(EPAD, 999.0, np.float32)  # dst rel in window; 999 = pad
        pos = 0
        ptr = 0
        for hh in range(2):
            for ww in range(cfg.NW):
                n = counts[c, hh, ww]
                sl = slice(ptr, ptr + n)
                gidx[pos:pos + n] = s_r[sl] - hh * cfg.SPLIT
                rel[pos:pos + n] = (d_l[sl] % cfg.DW).astype(np.float32)
                ptr += n
                pos += int(nch[hh, ww]) * 128
        assert ptr == len(s_r)

        def wrap16(v):
            a = v.astype(np.int16).reshape(-1, 16).T.copy()
            return np.tile(a, (8, 1))

        def lanes(v):
            return v.reshape(-1, 128).T.copy()

        cores.append(dict(gidx=wrap16(gidx), rel=lanes(rel)))
    return dict(nch=nch, NCH=NCH, nch0=nch0, EPAD=EPAD,
                chunk_meta=chunk_meta, cores=cores)


def call_schedule(cfg, plan):
    """Gather-call and tanh-segment schedule: calls of <=CSUP chunks that
    never span the int16 half boundary; segments of <=SEGN chunks of one
    (half, window) group within a call."""
    meta = plan["chunk_meta"]
    NCH, nch0 = plan["NCH"], plan["nch0"]
    calls = []
    c0 = 0
    while c0 < NCH:
        n = min(cfg.CSUP, NCH - c0)
        if meta[c0][0] == 0 and meta[c0 + n - 1][0] == 1:
            n = nch0 - c0
        segs = []
        j = 0
        while j < n:
            hh, ww = meta[c0 + j][0], meta[c0 + j][1]
            k = 1
            while (j + k < n and k < cfg.SEGN
                   and meta[c0 + j + k][0] == hh and meta[c0 + j + k][1] == ww):
                k += 1
            segs.append((j, k, ww))
            j += k
        calls.append((c0, n, meta[c0][0], segs))
        c0 += n
    return calls


def shard_inputs(cfg, inputs, plan):
    """Build per-core in_maps from full inputs."""
    x = np.asarray(inputs["x"], np.float32)
    ei = np.asarray(inputs["edge_index"])
    t1_w = np.asarray(inputs["t1_w"], np.float32)
    t1_b = np.asarray(inputs["t1_b"], np.float32)
    t2_w = np.asarray(inputs["t2_w"], np.float32)
    t2_b = np.asarray(inputs["t2_b"], np.float32)
    att_l = np.asarray(inputs["att_l"], np.float32)
    att_r = np.asarray(inputs["att_r"], np.float32)

    deg_all = np.bincount(ei[1].astype(np.int64), minlength=cfg.N).astype(np.float32)
    dinv_all = np.where(deg_all > 0, 1.0 / np.sqrt(np.maximum(deg_all, 1.0)), 0.0)

    w1t = t1_w.T.copy()                      # [IN, H]
    w1t_tiles = w1t.reshape(cfg.KT, 128, cfg.H)
    b1col = t1_b[:, None].copy()             # [H, 1]: per-partition bias
    bf = ml_dtypes.bfloat16
    # att vectors as [128, NL*2] (al, ar columns per layer)
    attlr = np.zeros((128, cfg.NL * 2), np.float32)
    for i in range(cfg.NL):
        attlr[:, 2 * i] = att_l[i]
        attlr[:, 2 * i + 1] = att_r[i]
    attlr = attlr.astype(bf)
    t2wt = t2_w.T.astype(bf)                 # [H, OUT]
    b2rep = np.broadcast_to(t2_b, (128, cfg.OUT)).copy()
    iota = np.tile(np.arange(cfg.DW, dtype=np.float32), (128, 1)).astype(bf)
    blockpat = np.kron(np.eye(cfg.SEGN, dtype=np.float32),
                       np.ones((1, 128), np.float32)).astype(bf)

    in_maps = []
    for c in range(cfg.NC):
        lo = c * cfg.NV
        xc = np.zeros((cfg.NP, cfg.IN), np.float32)
        xc[:cfg.NV] = x[lo:lo + cfg.NV]
        # [w, in-within-ktile, ktile, node] so SBUF partitions = in dim
        xt = xc.reshape(cfg.W, 128, cfg.KT, 128).transpose(0, 3, 2, 1).copy()
        # xt[w, i, k, n] = x[w*128+n, k*128+i]: partitions = in dim
        dv = np.zeros(cfg.NP, np.float32)
        dv[:cfg.NV] = dinv_all[lo:lo + cfg.NV]
        dinv_rep = np.broadcast_to(dv, (128, cfg.NP)).copy()
        pc = plan["cores"][c]
        in_maps.append(dict(
            xt=xt, dinv_rep=dinv_rep,
            w1t=w1t_tiles, b1col=b1col, attlr=attlr,
            t2wt=t2wt, b2rep=b2rep, iota=iota, blockpat=blockpat,
            gidx=pc["gidx"], rel=pc["rel"],
        ))
    return in_maps


# ----------------------------------------------------------------- builder

def build_program(cfg, plan, skip=frozenset()):
    NCH = plan["NCH"]
    meta = plan["chunk_meta"]
    EPAD = plan["EPAD"]
    W = cfg.W
    SEGN = cfg.SEGN
    calls = call_schedule(cfg, plan)

    nc = bacc.Bacc("TRN2", target_bir_lowering=False, debug=False,
                   num_devices=cfg.NC, num_swdge_queues=2,
                   dynamic_dma_scratch_size=cfg.DDSS)

    # ---- I/O
    t_xt = nc.dram_tensor("xt", [W, 128, cfg.KT, 128], F32, kind="ExternalInput")
    t_dinv = nc.dram_tensor("dinv_rep", [128, cfg.NP], F32, kind="ExternalInput")
    t_w1t = nc.dram_tensor("w1t", [cfg.KT, 128, cfg.H], F32, kind="ExternalInput")
    t_b1 = nc.dram_tensor("b1col", [cfg.H, 1], F32, kind="ExternalInput")
    t_att = nc.dram_tensor("attlr", [128, cfg.NL * 2], BF16, kind="ExternalInput")
    t_t2 = nc.dram_tensor("t2wt", [cfg.H, cfg.OUT], BF16, kind="ExternalInput")
    t_b2 = nc.dram_tensor("b2rep", [128, cfg.OUT], F32, kind="ExternalInput")
    t_iota = nc.dram_tensor("iota", [128, cfg.DW], BF16, kind="ExternalInput")
    t_bp = nc.dram_tensor("blockpat", [SEGN, SEGN * 128], BF16, kind="ExternalInput")
    t_gidx = nc.dram_tensor("gidx", [128, EPAD // 16], I16, kind="ExternalInput")
    t_rel = nc.dram_tensor("rel", [128, NCH], F32, kind="ExternalInput")
    t_lsm = nc.dram_tensor("lsm", [cfg.NP, cfg.OUT], F32, kind="ExternalOutput")
    t_emb = nc.dram_tensor("emb", [cfg.NP, cfg.OUT], F32, kind="ExternalOutput")

    # ---- internal DRAM
    RWE = 256                                 # table row elems (512B bf16)
    d_tab_loc = nc.dram_tensor("tab_loc", [cfg.NP, RWE], BF16)
    d_tab_full = nc.dram_tensor("tab_full", [cfg.NPG, RWE], BF16,
                                addr_space="Shared")
    d_ar_loc = nc.dram_tensor("ar_loc", [cfg.NP], F32)

    CS = cfg.CSUP
    rg = [list(range(cfg.NC))]

    with tile.TileContext(nc) as tc:
        with (
            tc.tile_pool(name="const", bufs=1) as cp,
            tc.tile_pool(name="stage", bufs=4) as sp,
            tc.tile_pool(name="gath", bufs=cfg.GPB) as gp,
            tc.tile_pool(name="seg", bufs=cfg.OHB) as tp,
            tc.tile_pool(name="oh", bufs=cfg.OHB) as op,
            tc.tile_pool(name="small", bufs=4) as mp,
            tc.tile_pool(name="psum", bufs=2, space="PSUM") as pp,
            tc.tile_pool(name="psumtt", bufs=3, space="PSUM") as pq,
        ):
            # ---------- constants / persistent state
            w1 = cp.tile([128, cfg.KT, cfg.H], F32, tag="w1")
            nc.sync.dma_start(out=w1[:], in_=t_w1t[:].rearrange("k p h -> p k h"))
            b1 = cp.tile([cfg.H, 1], F32, tag="b1")
            nc.sync.dma_start(out=b1[:], in_=t_b1[:])
            attb = cp.tile([128, cfg.NL * 2], BF16, tag="attb")
            nc.sync.dma_start(out=attb[:], in_=t_att[:])
            t2w = cp.tile([cfg.H, cfg.OUT], BF16, tag="t2w")
            nc.sync.dma_start(out=t2w[:], in_=t_t2[:])
            b2 = cp.tile([128, cfg.OUT], F32, tag="b2")
            nc.sync.dma_start(out=b2[:], in_=t_b2[:])
            iotab = cp.tile([128, cfg.DW], BF16, tag="iotab")
            nc.sync.dma_start(out=iotab[:], in_=t_iota[:])
            bp = cp.tile([SEGN, SEGN * 128], BF16, tag="bp")
            nc.sync.dma_start(out=bp[:], in_=t_bp[:])
            gidx = cp.tile([128, EPAD // 16], I16, tag="gidx")
            nc.sync.dma_start(out=gidx[:], in_=t_gidx[:])
            rel = cp.tile([128, NCH], F32, tag="rel")
            nc.sync.dma_start(out=rel[:], in_=t_rel[:])
            dinv = cp.tile([128, cfg.NP], F32, tag="dinv")
            nc.sync.dma_start(out=dinv[:], in_=t_dinv[:])
            ones1f = cp.tile([1, 128], F32, tag="ones1f")
            nc.vector.memset(ones1f[:], 1.0)
            identb = cp.tile([128, 128], BF16, tag="identb")
            make_identity(nc, identb[:])

            h_sb = cp.tile([128, cfg.NP], BF16, tag="h")
            raw_sb = cp.tile([128, cfg.NP], BF16, tag="raw")
            acc_sb = cp.tile([128, cfg.NP], F32, tag="acc")
            albx = cp.tile([128, W], BF16, tag="albx")
            arc = cp.tile([128, W], F32, tag="arc")
            arcrow = cp.tile([1, cfg.NP], F32, tag="arcrow")

            # ---------- phase A: h = relu(x @ t1_w.T + b1)  (feature-major)
            AB = cfg.AB
            for t0 in range(0, W if "phasea" not in skip else 0, AB):
                nb = min(AB, W - t0)
                xa = gp.tile([128, AB * cfg.KT * 128], F32, tag="xa")
                nc.sync.dma_start(
                    out=xa[:, :nb * cfg.KT * 128],
                    in_=t_xt[t0:t0 + nb].rearrange("w p k n -> p w k n"))
                for ti in range(nb):
                    t = t0 + ti
                    ps = pp.tile([128, 128], F32, tag="pg")
                    for k in range(cfg.KT):
                        o = (ti * cfg.KT + k) * 128
                        nc.tensor.matmul(ps[:], lhsT=w1[:, k, :],
                                         rhs=xa[:, o:o + 128],
                                         start=(k == 0), stop=(k == cfg.KT - 1))
                    nc.scalar.activation(h_sb[:, t * 128:(t + 1) * 128], ps[:],
                                         mybir.ActivationFunctionType.Relu,
                                         bias=b1[:])
                    nc.vector.tensor_scalar_mul(
                        raw_sb[:, t * 128:(t + 1) * 128],
                        h_sb[:, t * 128:(t + 1) * 128], EPS)

            # ---------- layers
            for li in range(cfg.NL):
                # node-side: al, ar, hs -> tables
                for w in range(W if "nprep" not in skip else 0):
                    sl = slice(w * 128, (w + 1) * 128)
                    ps2 = pp.tile([128, 128], F32, tag="pg")
                    nc.tensor.matmul(ps2[:, 0:2], lhsT=h_sb[:, sl],
                                     rhs=attb[:, 2 * li:2 * li + 2],
                                     start=True, stop=True)
                    nc.vector.tensor_copy(albx[:, w:w + 1], ps2[:, 0:1])
                    nc.vector.tensor_copy(arc[:, w:w + 1], ps2[:, 1:2])
                    hst = sp.tile([128, 128], BF16, tag="hst")
                    nc.vector.tensor_tensor(out=hst[:], in0=h_sb[:, sl],
                                            in1=dinv[:, sl],
                                            op=mybir.AluOpType.mult)
                    psT = pp.tile([128, 128], BF16, tag="pt")
                    nc.tensor.transpose(out=psT[:], in_=hst[:], identity=identb[:])
                    hsn = sp.tile([128, 128], BF16, tag="hsn")
                    nc.scalar.activation(hsn[:], psT[:],
                                         mybir.ActivationFunctionType.Copy)
                    nc.sync.dma_start(
                        out=d_tab_loc[w * 128:(w + 1) * 128, :cfg.H], in_=hsn[:])
                if "nprep" not in skip:
                    with nc.allow_non_contiguous_dma(reason="node-col store"):
                        nc.sync.dma_start(
                            out=d_tab_loc[:, cfg.H:cfg.H + 1].rearrange(
                                "(t p) c -> p (t c)", p=128),
                            in_=albx[:])
                    with nc.allow_non_contiguous_dma(reason="ar-col store"):
                        nc.sync.dma_start(
                            out=d_ar_loc[:].rearrange("(t p) -> p t", p=128),
                            in_=arc[:])
                    nc.sync.dma_start(out=arcrow[:], in_=d_ar_loc[None, :])
                # collective
                if "ag" not in skip:
                    nc.gpsimd.collective_compute(
                        "AllGather", mybir.AluOpType.bypass, replica_groups=rg,
                        ins=[d_tab_loc[:]], outs=[d_tab_full[:]])

                # edge pass
                nc.vector.memset(acc_sb[:], 0.0)
                psw = None
                for ci, (c0, ncall, half, segs) in enumerate(calls):
                    ne = ncall * 128
                    ghs = gp.tile([128, CS * RWE], BF16, tag="ghs")
                    ghv = ghs[:].rearrange("p (c e) -> p c e", e=RWE)
                    tab_src = (d_tab_full[:] if half == 0
                               else d_tab_full[cfg.SPLIT:, :])
                    if "gather" not in skip:
                        nc.gpsimd.dma_gather(
                            out_ap=ghv[:, :ncall, :],
                            in_ap=tab_src, idxs_ap=gidx[:, c0 * 8:(c0 + ncall) * 8],
                            num_idxs=ne, num_idxs_reg=ne, elem_size=RWE,
                            queue_num=ci % 2)
                    if "chunk" in skip:
                        continue
                    for (j0, n, ww) in segs:
                        DW = cfg.DW
                        wsl = slice(ww * DW, (ww + 1) * DW)
                        tt = tp.tile([128, SEGN * DW], BF16, tag="tt")
                        if cfg.TTMODE == "rank" and n >= 2 and DW == 128:
                            # al columns -> alS rows; one wide tanh per seg
                            psal = pp.tile([128, 128], BF16, tag="pt")
                            nc.tensor.transpose(out=psal[:n, :],
                                                in_=ghv[:, j0:j0 + n, cfg.H],
                                                identity=identb[:])
                            alS = mp.tile([SEGN, 128], BF16, tag="alS")
                            nc.scalar.activation(
                                alS[:n, :], psal[:n, :],
                                mybir.ActivationFunctionType.Copy)
                            pstt = pq.tile([128, SEGN * 128], F32, tag="pstt")
                            nc.tensor.matmul(pstt[:, :n * 128],
                                             lhsT=alS[0:n, :],
                                             rhs=bp[0:n, :n * 128],
                                             start=True, stop=False)
                            nc.tensor.matmul(
                                pstt[:, :n * 128].rearrange(
                                    "p (c m) -> p c m", m=128),
                                lhsT=ones1f[:],
                                rhs=arcrow[0:1, wsl][:, None, :].broadcast_to(
                                    (1, n, 128)),
                                start=False, stop=True)
                            nc.scalar.activation(
                                tt[:, :n * 128], pstt[:, :n * 128],
                                mybir.ActivationFunctionType.Tanh)
                        else:
                            # psar = ones (x) ar_w; per-chunk tanh with
                            # per-edge al bias from the gather tile
                            pstt = pq.tile([128, SEGN * 128], F32, tag="pstt")
                            nc.tensor.matmul(pstt[:, 0:DW], lhsT=ones1f[:],
                                             rhs=arcrow[0:1, wsl],
                                             start=True, stop=True)
                            for j in range(n):
                                nc.scalar.activation(
                                    tt[:, j * DW:(j + 1) * DW], pstt[:, 0:DW],
                                    mybir.ActivationFunctionType.Tanh,
                                    bias=ghv[:, j0 + j, cfg.H:cfg.H + 1])
                        ohp = op.tile([128, SEGN * DW], BF16, tag="ohp")
                        for j in range(n):
                            cj = c0 + j0 + j
                            nc.vector.tensor_scalar(
                                ohp[:, j * DW:(j + 1) * DW], iotab[:],
                                rel[:, cj:cj + 1], None,
                                op0=mybir.AluOpType.is_equal)
                        ohm = op.tile([128, SEGN * DW], BF16, tag="ohm")
                        nc.vector.tensor_tensor(out=ohm[:, :n * DW],
                                                in0=ohp[:, :n * DW],
                                                in1=tt[:, :n * DW],
                                                op=mybir.AluOpType.mult)
                        for j in range(n):
                            cj = c0 + j0 + j
                            first, last = meta[cj][2], meta[cj][3]
                            if first:
                                psw = pp.tile([128, 128], F32, tag="pg")
                            nc.tensor.matmul(psw[:, 0:DW],
                                             lhsT=ghv[:, j0 + j, 0:cfg.H],
                                             rhs=ohm[:, j * DW:(j + 1) * DW],
                                             start=first, stop=last)
                            if last:
                                nc.vector.tensor_tensor(
                                    out=acc_sb[:, wsl], in0=acc_sb[:, wsl],
                                    in1=psw[:, 0:DW], op=mybir.AluOpType.add)
                # h_new = dinv * acc + raw  (raw = EPS*h0)
                if "chunk" not in skip:
                    nc.vector.tensor_tensor(out=acc_sb[:], in0=acc_sb[:],
                                            in1=dinv[:], op=mybir.AluOpType.mult)
                    nc.vector.tensor_add(h_sb[:], acc_sb[:], raw_sb[:])
            del psw

            # ---------- phase C: emb = h @ t2_w.T + b2; lsm = log_softmax
            # (all Exp first, one wide Ln at the end: no act-table thrash)
            shw = cp.tile([128, W * cfg.OUT], F32, tag="shw")
            smw = cp.tile([128, W], F32, tag="smw")
            for t in range(W if "phasec" not in skip else 0):
                sl = slice(t * 128, (t + 1) * 128)
                osl = slice(t * cfg.OUT, (t + 1) * cfg.OUT)
                pse = pp.tile([128, 128], F32, tag="pg")
                nc.tensor.matmul(pse[:, :cfg.OUT], lhsT=h_sb[:, sl], rhs=t2w[:],
                                 start=True, stop=True)
                emb = sp.tile([128, cfg.OUT], F32, tag="embt")
                nc.vector.tensor_add(emb[:], pse[:, :cfg.OUT], b2[:])
                nc.sync.dma_start(out=t_emb[t * 128:(t + 1) * 128, :], in_=emb[:])
                mx = mp.tile([128, 1], F32, tag="mx")
                nc.vector.tensor_reduce(mx[:], emb[:], axis=mybir.AxisListType.X,
                                        op=mybir.AluOpType.max)
                nc.vector.tensor_scalar(shw[:, osl], emb[:], mx[:], None,
                                        op0=mybir.AluOpType.subtract)
                ex = sp.tile([128, cfg.OUT], F32, tag="ex")
                nc.scalar.activation(ex[:], shw[:, osl],
                                     mybir.ActivationFunctionType.Exp)
                nc.vector.tensor_reduce(smw[:, t:t + 1], ex[:],
                                        axis=mybir.AxisListType.X,
                                        op=mybir.AluOpType.add)
            if "phasec" not in skip:
                nc.scalar.activation(smw[:], smw[:],
                                     mybir.ActivationFunctionType.Ln)
                for t in range(W):
                    osl = slice(t * cfg.OUT, (t + 1) * cfg.OUT)
                    lsm = sp.tile([128, cfg.OUT], F32, tag="lsmt")
                    nc.vector.tensor_scalar(lsm[:], shw[:, osl], smw[:, t:t + 1],
                                            None, op0=mybir.AluOpType.subtract)
                    nc.sync.dma_start(out=t_lsm[t * 128:(t + 1) * 128, :],
                                      in_=lsm[:])

    nc.finalize()
    return nc


# ------------------------------------------------------- cached PJRT runner

def _make_runner(nc, n_cores):
    """Like bass2jax.run_bass_via_pjrt, but builds the jitted executable once
    so repeated calls don't re-trace/re-compile."""
    import jax
    import concourse.mybir as mb
    from jax.sharding import Mesh, PartitionSpec
    from jax.experimental.shard_map import shard_map
    from concourse.bass2jax import (install_neuronx_cc_hook, partition_id_tensor,
                                    _bass_exec_p)
    install_neuronx_cc_hook()
    partition_name = nc.partition_id_tensor.name if nc.partition_id_tensor else None
    in_names, out_names, out_avals, zero_outs = [], [], [], []
    for alloc in nc.m.functions[0].allocations:
        if not isinstance(alloc, mb.MemoryLocationSet):
            continue
        name = alloc.memorylocations[0].name
        if alloc.kind == "ExternalInput":
            if name != partition_name:
                in_names.append(name)
        elif alloc.kind == "ExternalOutput":
            out_names.append(name)
            shape = tuple(alloc.tensor_shape)
            dtype = mb.dt.np(alloc.dtype)
            out_avals.append(jax.core.ShapedArray(shape, dtype))
            zero_outs.append(np.zeros(shape, dtype))
    n_params = len(in_names)
    n_outs = len(out_avals)
    all_in_names = list(in_names) + list(out_names)
    if partition_name is not None:
        all_in_names.append(partition_name)
    donate = tuple(range(n_params, n_params + n_outs))

    def _body(*args):
        operands = list(args)
        if partition_name is not None:
            operands.append(partition_id_tensor())
        return tuple(_bass_exec_p.bind(
            *operands, out_avals=tuple(out_avals), in_names=tuple(all_in_names),
            out_names=tuple(out_names), lowering_input_output_aliases=(),
            sim_require_finite=True, sim_require_nnan=True, nc=nc))

    devices = jax.devices()[:n_cores]
    mesh = Mesh(np.asarray(devices), ("core",))
    in_specs = (PartitionSpec("core"),) * (n_params + n_outs)
    out_specs = (PartitionSpec("core"),) * n_outs
    sharded = jax.jit(
        shard_map(_body, mesh=mesh, in_specs=in_specs, out_specs=out_specs,
                  check_rep=False),
        donate_argnums=donate, keep_unused=True)

    def call(in_maps):
        concat_in = [
            np.concatenate([np.asarray(in_maps[c][k]) for c in range(n_cores)], 0)
            for k in in_names
        ]
        concat_zeros = [
            np.zeros((n_cores * z.shape[0], *z.shape[1:]), z.dtype)
            for z in zero_outs
        ]
        out_arrs = sharded(*concat_in, *concat_zeros)
        jax.block_until_ready(out_arrs)
        return [
            {k: np.asarray(out_arrs[i]).reshape(n_cores, *out_avals[i].shape)[c]
             for i, k in enumerate(out_names)}
            for c in range(n_cores)
        ]

    return call


# TimelineSim(non-collective 859,073) + rust collective cost model
# (3 x 283,747 for the 25.7MB AllGather); see t_sim.py. The same model
# reproduced the previous kernel's differencing estimate within 1%.
HW_EXEC_NS_ESTIMATE = 1710313

# ----------------------------------------------------------------- entry

_CACHE = {}


def run(cfg, inputs, trace=False):
    ei = np.asarray(inputs["edge_index"])
    key = (cfg.N, cfg.E, cfg.NL, hash(ei.tobytes()))
    if key in _CACHE:
        runner, plan = _CACHE[key]
    else:
        plan = plan_edges(cfg, ei)
        nc = build_program(cfg, plan)
        runner = _make_runner(nc, cfg.NC)
        _CACHE[key] = (runner, plan)
    in_maps = shard_inputs(cfg, inputs, plan)
    results = runner(in_maps)
    lsm = np.concatenate([results[c]["lsm"][:cfg.NV] for c in range(cfg.NC)], 0)
    emb = np.concatenate([results[c]["emb"][:cfg.NV] for c in range(cfg.NC)], 0)
    return (lsm, emb), None


def kernel(**inputs):
    (lsm, emb), _ = run(FULL, inputs)
    return lsm, emb


# revision 91
# speedup vs baseline: 1.0246x; 1.0246x over previous
"""FAGCN (FAConv x3) Trainium2 kernel, 8-core SPMD — v2.

Sharding: nodes partitioned across 8 cores (6250 each, padded to 6272).
Edges assigned to the owner of dst. Node state is kept FEATURE-MAJOR
([128 feat, nodes]) so phase A/C need no transposes and per-window
al/ar are PE matvecs.

Per layer: each core computes its table slice (rows = [h*dinv (128
bf16) | al | pad] = 512B), AllGathers the table, then runs an edge
pass over 128-edge chunks grouped by 64-node dst window (64 halves
the tanh/one-hot/segsum widths on the bottleneck Act engine; the
~14% extra chunk padding costs less than it saves):
  - dma_gather of rows by src (two int16 halves; <=1024 descriptors
    per call -- the SWDGE ring hangs the device beyond that),
  - per segment (<=4 chunks, one dst window): rank-1 PE matmul
    broadcasts ar_w into PSUM; per chunk one Act tanh with the
    per-edge al bias read straight from the gathered row -> tt,
  - per chunk: one-hot tensor_scalar (is_equal, 4x bf16 mode); per
    segment a wide tensor_tensor multiply by tt (2x bf16 mode) and
    per chunk a TensorE segment-sum matmul accumulating [feat, dst]
    in PSUM (lhsT=gathered rows, rhs=one-hot),
  - group end: DVE adds PSUM into the wide accumulator.
h_new = dinv * acc + EPS*h0 as two wide DVE ops per layer.
Phase C batches all Exp then one wide Ln (activation-table loads are
1.3us each; Relu/Tanh/Copy/Exp share one table set, Ln does not).
"""
import ml_dtypes
import numpy as np

import concourse.bacc as bacc
import concourse.bass as bass
import concourse.mybir as mybir
import concourse.tile as tile
from concourse.bass_utils import run_bass_kernel_spmd
from concourse.masks import make_identity

F32 = mybir.dt.float32
BF16 = mybir.dt.bfloat16
I16 = mybir.dt.int16

EPS = 0.1


class Cfg:
    def __init__(self, n_nodes, n_edges, in_dim, out_dim, n_layers,
                 n_cores=8, split=32768, csup=8, segn=4, bf16=True):
        self.BF16 = bf16
        self.N = n_nodes
        self.E = n_edges
        self.IN = in_dim
        self.H = 128
        self.OUT = out_dim
        self.NL = n_layers
        self.NC = n_cores
        self.NV = n_nodes // n_cores          # owned nodes per core
        assert self.NV * n_cores == n_nodes
        self.W = (self.NV + 127) // 128       # dst windows per core
        self.NP = self.W * 128                # padded nodes per core
        self.NPG = self.NP * n_cores          # padded global nodes
        self.KT = in_dim // 128               # k-tiles of the input matmul
        assert in_dim % 128 == 0
        self.SPLIT = split                    # int16 table-half boundary
        self.CSUP = csup                      # chunks per gather call
        self.DW = 64                          # dst-window width (nodes)
        self.NW = self.NP // self.DW          # dst windows per core
        self.SEGN = segn                      # chunks per tanh segment
        self.AB = 2                           # node-tiles per x load
        self.GPB = 6                          # gather pool bufs
        self.OHB = 6                          # one-hot / tt pool bufs
        self.TTMODE = "bias"                  # "rank" | "bias" tanh build
        # SWDGE descriptor ring (carved from SBUF): csup*128 descs * 16B.
        # >1024 descs per gather hangs the device at the default 16 KiB.
        self.DDSS = max(16384, csup * 128 * 16)


FULL = Cfg(50000, 600000, 512, 64, 3)


# ----------------------------------------------------------------- planner

def plan_edges(cfg, edge_index):
    """Host-side edge sharding. Returns the uniform chunk schedule and the
    per-core packed arrays."""
    src = edge_index[0].astype(np.int64)
    dst = edge_index[1].astype(np.int64)
    owner = dst // cfg.NV
    remap = (src // cfg.NV) * cfg.NP + (src % cfg.NV)   # padded global row id

    per_core = []
    counts = np.zeros((cfg.NC, 2, cfg.NW), np.int64)
    for c in range(cfg.NC):
        m = owner == c
        s_r = remap[m]
        d_l = dst[m] - c * cfg.NV
        w = d_l // cfg.DW
        h = (s_r >= cfg.SPLIT).astype(np.int64)
        order = np.lexsort((d_l, w, h))
        s_r, d_l, w, h = s_r[order], d_l[order], w[order], h[order]
        for hh in range(2):
            for ww in range(cfg.NW):
                counts[c, hh, ww] = np.count_nonzero((h == hh) & (w == ww))
        per_core.append((s_r, d_l, h, w))

    nch = np.maximum((counts.max(axis=0) + 127) // 128, 0)  # [2, W] chunks
    nch[counts.max(axis=0) == 0] = 0
    NCH = int(nch.sum())
    EPAD = NCH * 128

    # global chunk meta: (half, window, first_of_group, last_of_group)
    chunk_meta = []
    for hh in range(2):
        for ww in range(cfg.NW):
            n = int(nch[hh, ww])
            for k in range(n):
                chunk_meta.append((hh, ww, k == 0, k == n - 1))
    nch0 = int(nch[0].sum())   # chunks in half 0

    cores = []
    for c in range(cfg.NC):
        s_r, d_l, h, w = per_core[c]
        gidx = np.zeros(EPAD, np.int64)     # table row (half-rebased)
        rel = np.full(EPAD, 999.0, np.float32)  # dst rel in window; 999 = pad
        pos = 0
        ptr = 0
        for hh in range(2):
            for ww in range(cfg.NW):
                n = counts[c, hh, ww]
                sl = slice(ptr, ptr + n)
                gidx[pos:pos + n] = s_r[sl] - hh * cfg.SPLIT
                rel[pos:pos + n] = (d_l[sl] % cfg.DW).astype(np.float32)
                ptr += n
                pos += int(nch[hh, ww]) * 128
        assert ptr == len(s_r)

        def wrap16(v):
            a = v.astype(np.int16).reshape(-1, 16).T.copy()
            return np.tile(a, (8, 1))

        def lanes(v):
            return v.reshape(-1, 128).T.copy()

        cores.append(dict(gidx=wrap16(gidx), rel=lanes(rel)))
    return dict(nch=nch, NCH=NCH, nch0=nch0, EPAD=EPAD,
                chunk_meta=chunk_meta, cores=cores)


def call_schedule(cfg, plan):
    """Gather-call and tanh-segment schedule: calls of <=CSUP chunks that
    never span the int16 half boundary; segments of <=SEGN chunks of one
    (half, window) group within a call."""
    meta = plan["chunk_meta"]
    NCH, nch0 = plan["NCH"], plan["nch0"]
    calls = []
    c0 = 0
    while c0 < NCH:
        n = min(cfg.CSUP, NCH - c0)
        if meta[c0][0] == 0 and meta[c0 + n - 1][0] == 1:
            n = nch0 - c0
        segs = []
        j = 0
        while j < n:
            hh, ww = meta[c0 + j][0], meta[c0 + j][1]
            k = 1
            while (j + k < n and k < cfg.SEGN
                   and meta[c0 + j + k][0] == hh and meta[c0 + j + k][1] == ww):
                k += 1
            segs.append((j, k, ww))
            j += k
        calls.append((c0, n, meta[c0][0], segs))
        c0 += n
    return calls


def shard_inputs(cfg, inputs, plan):
    """Build per-core in_maps from full inputs."""
    x = np.asarray(inputs["x"], np.float32)
    ei = np.asarray(inputs["edge_index"])
    t1_w = np.asarray(inputs["t1_w"], np.float32)
    t1_b = np.asarray(inputs["t1_b"], np.float32)
    t2_w = np.asarray(inputs["t2_w"], np.float32)
    t2_b = np.asarray(inputs["t2_b"], np.float32)
    att_l = np.asarray(inputs["att_l"], np.float32)
    att_r = np.asarray(inputs["att_r"], np.float32)

    deg_all = np.bincount(ei[1].astype(np.int64), minlength=cfg.N).astype(np.float32)
    dinv_all = np.where(deg_all > 0, 1.0 / np.sqrt(np.maximum(deg_all, 1.0)), 0.0)

    w1t = t1_w.T.copy()                      # [IN, H]
    w1t_tiles = w1t.reshape(cfg.KT, 128, cfg.H)
    b1col = t1_b[:, None].copy()             # [H, 1]: per-partition bias
    bf = ml_dtypes.bfloat16
    # att vectors as [128, NL*2] (al, ar columns per layer)
    attlr = np.zeros((128, cfg.NL * 2), np.float32)
    for i in range(cfg.NL):
        attlr[:, 2 * i] = att_l[i]
        attlr[:, 2 * i + 1] = att_r[i]
    attlr = attlr.astype(bf)
    t2wt = t2_w.T.astype(bf)                 # [H, OUT]
    b2rep = np.broadcast_to(t2_b, (128, cfg.OUT)).copy()
    iota = np.tile(np.arange(cfg.DW, dtype=np.float32), (128, 1)).astype(bf)
    blockpat = np.kron(np.eye(cfg.SEGN, dtype=np.float32),
                       np.ones((1, 128), np.float32)).astype(bf)

    in_maps = []
    for c in range(cfg.NC):
        lo = c * cfg.NV
        xc = np.zeros((cfg.NP, cfg.IN), np.float32)
        xc[:cfg.NV] = x[lo:lo + cfg.NV]
        # [w, in-within-ktile, ktile, node] so SBUF partitions = in dim
        xt = xc.reshape(cfg.W, 128, cfg.KT, 128).transpose(0, 3, 2, 1).copy()
        # xt[w, i, k, n] = x[w*128+n, k*128+i]: partitions = in dim
        dv = np.zeros(cfg.NP, np.float32)
        dv[:cfg.NV] = dinv_all[lo:lo + cfg.NV]
        dinv_rep = np.broadcast_to(dv, (128, cfg.NP)).copy()
        pc = plan["cores"][c]
        in_maps.append(dict(
            xt=xt, dinv_rep=dinv_rep,
            w1t=w1t_tiles, b1col=b1col, attlr=attlr,
            t2wt=t2wt, b2rep=b2rep, iota=iota, blockpat=blockpat,
            gidx=pc["gidx"], rel=pc["rel"],
        ))
    return in_maps


# ----------------------------------------------------------------- builder

def build_program(cfg, plan, skip=frozenset()):
    NCH = plan["NCH"]
    meta = plan["chunk_meta"]
    EPAD = plan["EPAD"]
    W = cfg.W
    SEGN = cfg.SEGN
    calls = call_schedule(cfg, plan)

    nc = bacc.Bacc("TRN2", target_bir_lowering=False, debug=False,
                   num_devices=cfg.NC, num_swdge_queues=2,
                   dynamic_dma_scratch_size=cfg.DDSS)

    # ---- I/O
    t_xt = nc.dram_tensor("xt", [W, 128, cfg.KT, 128], F32, kind="ExternalInput")
    t_dinv = nc.dram_tensor("dinv_rep", [128, cfg.NP], F32, kind="ExternalInput")
    t_w1t = nc.dram_tensor("w1t", [cfg.KT, 128, cfg.H], F32, kind="ExternalInput")
    t_b1 = nc.dram_tensor("b1col", [cfg.H, 1], F32, kind="ExternalInput")
    t_att = nc.dram_tensor("attlr", [128, cfg.NL * 2], BF16, kind="ExternalInput")
    t_t2 = nc.dram_tensor("t2wt", [cfg.H, cfg.OUT], BF16, kind="ExternalInput")
    t_b2 = nc.dram_tensor("b2rep", [128, cfg.OUT], F32, kind="ExternalInput")
    t_iota = nc.dram_tensor("iota", [128, cfg.DW], BF16, kind="ExternalInput")
    t_bp = nc.dram_tensor("blockpat", [SEGN, SEGN * 128], BF16, kind="ExternalInput")
    t_gidx = nc.dram_tensor("gidx", [128, EPAD // 16], I16, kind="ExternalInput")
    t_rel = nc.dram_tensor("rel", [128, NCH], F32, kind="ExternalInput")
    t_lsm = nc.dram_tensor("lsm", [cfg.NP, cfg.OUT], F32, kind="ExternalOutput")
    t_emb = nc.dram_tensor("emb", [cfg.NP, cfg.OUT], F32, kind="ExternalOutput")

    # ---- internal DRAM
    RWE = 256                                 # table row elems (512B bf16)
    d_tab_loc = nc.dram_tensor("tab_loc", [cfg.NP, RWE], BF16)
    d_tab_full = nc.dram_tensor("tab_full", [cfg.NPG, RWE], BF16,
                                addr_space="Shared")
    d_ar_loc = nc.dram_tensor("ar_loc", [cfg.NP], F32)

    CS = cfg.CSUP
    rg = [list(range(cfg.NC))]

    with tile.TileContext(nc) as tc:
        with (
            tc.tile_pool(name="const", bufs=1) as cp,
            tc.tile_pool(name="stage", bufs=4) as sp,
            tc.tile_pool(name="gath", bufs=cfg.GPB) as gp,
            tc.tile_pool(name="seg", bufs=cfg.OHB) as tp,
            tc.tile_pool(name="oh", bufs=cfg.OHB) as op,
            tc.tile_pool(name="small", bufs=4) as mp,
            tc.tile_pool(name="psum", bufs=2, space="PSUM") as pp,
            tc.tile_pool(name="psumtt", bufs=3, space="PSUM") as pq,
        ):
            # ---------- constants / persistent state
            w1 = cp.tile([128, cfg.KT, cfg.H], F32, tag="w1")
            nc.sync.dma_start(out=w1[:], in_=t_w1t[:].rearrange("k p h -> p k h"))
            b1 = cp.tile([cfg.H, 1], F32, tag="b1")
            nc.sync.dma_start(out=b1[:], in_=t_b1[:])
            attb = cp.tile([128, cfg.NL * 2], BF16, tag="attb")
            nc.sync.dma_start(out=attb[:], in_=t_att[:])
            t2w = cp.tile([cfg.H, cfg.OUT], BF16, tag="t2w")
            nc.sync.dma_start(out=t2w[:], in_=t_t2[:])
            b2 = cp.tile([128, cfg.OUT], F32, tag="b2")
            nc.sync.dma_start(out=b2[:], in_=t_b2[:])
            iotab = cp.tile([128, cfg.DW], BF16, tag="iotab")
            nc.sync.dma_start(out=iotab[:], in_=t_iota[:])
            bp = cp.tile([SEGN, SEGN * 128], BF16, tag="bp")
            nc.sync.dma_start(out=bp[:], in_=t_bp[:])
            gidx = cp.tile([128, EPAD // 16], I16, tag="gidx")
            nc.sync.dma_start(out=gidx[:], in_=t_gidx[:])
            rel = cp.tile([128, NCH], F32, tag="rel")
            nc.sync.dma_start(out=rel[:], in_=t_rel[:])
            dinv = cp.tile([128, cfg.NP], F32, tag="dinv")
            nc.sync.dma_start(out=dinv[:], in_=t_dinv[:])
            ones1f = cp.tile([1, 128], F32, tag="ones1f")
            nc.vector.memset(ones1f[:], 1.0)
            identb = cp.tile([128, 128], BF16, tag="identb")
            make_identity(nc, identb[:])

            h_sb = cp.tile([128, cfg.NP], BF16, tag="h")
            raw_sb = cp.tile([128, cfg.NP], BF16, tag="raw")
            acc_sb = cp.tile([128, cfg.NP], F32, tag="acc")
            albx = cp.tile([128, W], BF16, tag="albx")
            arc = cp.tile([128, W], F32, tag="arc")
            arcrow = cp.tile([1, cfg.NP], F32, tag="arcrow")

            # ---------- phase A: h = relu(x @ t1_w.T + b1)  (feature-major)
            AB = cfg.AB
            for t0 in range(0, W if "phasea" not in skip else 0, AB):
                nb = min(AB, W - t0)
                xa = gp.tile([128, AB * cfg.KT * 128], F32, tag="xa")
                nc.sync.dma_start(
                    out=xa[:, :nb * cfg.KT * 128],
                    in_=t_xt[t0:t0 + nb].rearrange("w p k n -> p w k n"))
                for ti in range(nb):
                    t = t0 + ti
                    ps = pp.tile([128, 128], F32, tag="pg")
                    for k in range(cfg.KT):
                        o = (ti * cfg.KT + k) * 128
                        nc.tensor.matmul(ps[:], lhsT=w1[:, k, :],
                                         rhs=xa[:, o:o + 128],
                                         start=(k == 0), stop=(k == cfg.KT - 1))
                    nc.scalar.activation(h_sb[:, t * 128:(t + 1) * 128], ps[:],
                                         mybir.ActivationFunctionType.Relu,
                                         bias=b1[:])
                    nc.vector.tensor_scalar_mul(
                        raw_sb[:, t * 128:(t + 1) * 128],
                        h_sb[:, t * 128:(t + 1) * 128], EPS)

            # ---------- per-window helper blocks
            def nprep_win(w, li):
                sl = slice(w * 128, (w + 1) * 128)
                ps2 = pp.tile([128, 128], F32, tag="pg")
                nc.tensor.matmul(ps2[:, 0:2], lhsT=h_sb[:, sl],
                                 rhs=attb[:, 2 * li:2 * li + 2],
                                 start=True, stop=True)
                nc.vector.tensor_copy(albx[:, w:w + 1], ps2[:, 0:1])
                nc.vector.tensor_copy(arc[:, w:w + 1], ps2[:, 1:2])
                hst = sp.tile([128, 128], BF16, tag="hst")
                nc.vector.tensor_tensor(out=hst[:], in0=h_sb[:, sl],
                                        in1=dinv[:, sl],
                                        op=mybir.AluOpType.mult)
                psT = pp.tile([128, 128], BF16, tag="pt")
                nc.tensor.transpose(out=psT[:], in_=hst[:], identity=identb[:])
                hsn = sp.tile([128, 128], BF16, tag="hsn")
                nc.scalar.activation(hsn[:], psT[:],
                                     mybir.ActivationFunctionType.Copy)
                nc.sync.dma_start(
                    out=d_tab_loc[w * 128:(w + 1) * 128, :cfg.H], in_=hsn[:])

            shw = cp.tile([128, W * cfg.OUT], F32, tag="shw")
            smw = cp.tile([128, W], F32, tag="smw")

            def phasec_win(t):
                sl = slice(t * 128, (t + 1) * 128)
                osl = slice(t * cfg.OUT, (t + 1) * cfg.OUT)
                pse = pp.tile([128, 128], F32, tag="pg")
                nc.tensor.matmul(pse[:, :cfg.OUT], lhsT=h_sb[:, sl], rhs=t2w[:],
                                 start=True, stop=True)
                emb = sp.tile([128, cfg.OUT], F32, tag="embt")
                nc.vector.tensor_add(emb[:], pse[:, :cfg.OUT], b2[:])
                nc.sync.dma_start(out=t_emb[t * 128:(t + 1) * 128, :], in_=emb[:])
                mx = mp.tile([128, 1], F32, tag="mx")
                nc.vector.tensor_reduce(mx[:], emb[:], axis=mybir.AxisListType.X,
                                        op=mybir.AluOpType.max)
                nc.vector.tensor_scalar(shw[:, osl], emb[:], mx[:], None,
                                        op0=mybir.AluOpType.subtract)
                ex = sp.tile([128, cfg.OUT], F32, tag="ex")
                nc.scalar.activation(ex[:], shw[:, osl],
                                     mybir.ActivationFunctionType.Exp)
                nc.vector.tensor_reduce(smw[:, t:t + 1], ex[:],
                                        axis=mybir.AxisListType.X,
                                        op=mybir.AluOpType.add)

            def hnew_win(w):
                sl = slice(w * 128, (w + 1) * 128)
                nc.vector.tensor_tensor(out=h_sb[:, sl], in0=acc_sb[:, sl],
                                        in1=dinv[:, sl],
                                        op=mybir.AluOpType.mult)
                nc.vector.tensor_add(h_sb[:, sl], h_sb[:, sl], raw_sb[:, sl])

            # final chunk index of each 128-node window: the last chunk among
            # its four (half, 64-window) groups, in chunk order
            lastchunk2win = {}
            glast = {}
            for cix, (hh, ww, first, last) in enumerate(meta):
                if last:
                    glast[(hh, ww)] = cix
            for w in range(W):
                cands = [glast[g] for g in
                         ((0, 2 * w), (0, 2 * w + 1), (1, 2 * w), (1, 2 * w + 1))
                         if g in glast]
                lastchunk2win[max(cands)] = w

            # ---------- layers
            for li in range(cfg.NL):
                # node-side: al, ar, hs -> tables (layer 0: standalone; later
                # layers interleave into the previous edge pass's tail)
                if li == 0:
                    for w in range(W if "nprep" not in skip else 0):
                        nprep_win(w, 0)
                if "nprep" not in skip:
                    with nc.allow_non_contiguous_dma(reason="node-col store"):
                        nc.sync.dma_start(
                            out=d_tab_loc[:, cfg.H:cfg.H + 1].rearrange(
                                "(t p) c -> p (t c)", p=128),
                            in_=albx[:])
                    with nc.allow_non_contiguous_dma(reason="ar-col store"):
                        nc.sync.dma_start(
                            out=d_ar_loc[:].rearrange("(t p) -> p t", p=128),
                            in_=arc[:])
                    nc.sync.dma_start(out=arcrow[:], in_=d_ar_loc[None, :])
                # collective
                if "ag" not in skip:
                    nc.gpsimd.collective_compute(
                        "AllGather", mybir.AluOpType.bypass, replica_groups=rg,
                        ins=[d_tab_loc[:]], outs=[d_tab_full[:]])

                # edge pass
                nc.vector.memset(acc_sb[:], 0.0)
                psw = None
                for ci, (c0, ncall, half, segs) in enumerate(calls):
                    ne = ncall * 128
                    ghs = gp.tile([128, CS * RWE], BF16, tag="ghs")
                    ghv = ghs[:].rearrange("p (c e) -> p c e", e=RWE)
                    tab_src = (d_tab_full[:] if half == 0
                               else d_tab_full[cfg.SPLIT:, :])
                    if "gather" not in skip:
                        nc.gpsimd.dma_gather(
                            out_ap=ghv[:, :ncall, :],
                            in_ap=tab_src, idxs_ap=gidx[:, c0 * 8:(c0 + ncall) * 8],
                            num_idxs=ne, num_idxs_reg=ne, elem_size=RWE,
                            queue_num=ci % 2)
                    if "chunk" in skip:
                        continue
                    for (j0, n, ww) in segs:
                        DW = cfg.DW
                        wsl = slice(ww * DW, (ww + 1) * DW)
                        tt = tp.tile([128, SEGN * DW], BF16, tag="tt")
                        if cfg.TTMODE == "rank" and n >= 2 and DW == 128:
                            # al columns -> alS rows; one wide tanh per seg
                            psal = pp.tile([128, 128], BF16, tag="pt")
                            nc.tensor.transpose(out=psal[:n, :],
                                                in_=ghv[:, j0:j0 + n, cfg.H],
                                                identity=identb[:])
                            alS = mp.tile([SEGN, 128], BF16, tag="alS")
                            nc.scalar.activation(
                                alS[:n, :], psal[:n, :],
                                mybir.ActivationFunctionType.Copy)
                            pstt = pq.tile([128, SEGN * 128], F32, tag="pstt")
                            nc.tensor.matmul(pstt[:, :n * 128],
                                             lhsT=alS[0:n, :],
                                             rhs=bp[0:n, :n * 128],
                                             start=True, stop=False)
                            nc.tensor.matmul(
                                pstt[:, :n * 128].rearrange(
                                    "p (c m) -> p c m", m=128),
                                lhsT=ones1f[:],
                                rhs=arcrow[0:1, wsl][:, None, :].broadcast_to(
                                    (1, n, 128)),
                                start=False, stop=True)
                            nc.scalar.activation(
                                tt[:, :n * 128], pstt[:, :n * 128],
                                mybir.ActivationFunctionType.Tanh)
                        else:
                            # psar = ones (x) ar_w; per-chunk tanh with
                            # per-edge al bias from the gather tile
                            pstt = pq.tile([128, SEGN * 128], F32, tag="pstt")
                            nc.tensor.matmul(pstt[:, 0:DW], lhsT=ones1f[:],
                                             rhs=arcrow[0:1, wsl],
                                             start=True, stop=True)
                            for j in range(n):
                                nc.scalar.activation(
                                    tt[:, j * DW:(j + 1) * DW], pstt[:, 0:DW],
                                    mybir.ActivationFunctionType.Tanh,
                                    bias=ghv[:, j0 + j, cfg.H:cfg.H + 1])
                        ohp = op.tile([128, SEGN * DW], BF16, tag="ohp")
                        for j in range(n):
                            cj = c0 + j0 + j
                            nc.vector.tensor_scalar(
                                ohp[:, j * DW:(j + 1) * DW], iotab[:],
                                rel[:, cj:cj + 1], None,
                                op0=mybir.AluOpType.is_equal)
                        ohm = op.tile([128, SEGN * DW], BF16, tag="ohm")
                        nc.vector.tensor_tensor(out=ohm[:, :n * DW],
                                                in0=ohp[:, :n * DW],
                                                in1=tt[:, :n * DW],
                                                op=mybir.AluOpType.mult)
                        for j in range(n):
                            cj = c0 + j0 + j
                            first, last = meta[cj][2], meta[cj][3]
                            if first:
                                psw = pp.tile([128, 128], F32, tag="pg")
                            nc.tensor.matmul(psw[:, 0:DW],
                                             lhsT=ghv[:, j0 + j, 0:cfg.H],
                                             rhs=ohm[:, j * DW:(j + 1) * DW],
                                             start=first, stop=last)
                            if last:
                                nc.vector.tensor_tensor(
                                    out=acc_sb[:, wsl], in0=acc_sb[:, wsl],
                                    in1=psw[:, 0:DW], op=mybir.AluOpType.add)
                                wfin = lastchunk2win.get(cj)
                                if wfin is not None:
                                    # node-window final: h_new, then next
                                    # layer's nprep (or phase C) rides the
                                    # edge-pass tail
                                    hnew_win(wfin)
                                    if li + 1 < cfg.NL:
                                        if "nprep" not in skip:
                                            nprep_win(wfin, li + 1)
                                    elif "phasec" not in skip:
                                        phasec_win(wfin)
            del psw

            # ---------- phase C epilogue: one wide Ln, then lsm stores
            # (per-window Exp blocks ran inline in the last edge pass)
            if "phasec" not in skip:
                nc.scalar.activation(smw[:], smw[:],
                                     mybir.ActivationFunctionType.Ln)
                for t in range(W):
                    osl = slice(t * cfg.OUT, (t + 1) * cfg.OUT)
                    lsm = sp.tile([128, cfg.OUT], F32, tag="lsmt")
                    nc.vector.tensor_scalar(lsm[:], shw[:, osl], smw[:, t:t + 1],
                                            None, op0=mybir.AluOpType.subtract)
                    nc.sync.dma_start(out=t_lsm[t * 128:(t + 1) * 128, :],
                                      in_=lsm[:])

    nc.finalize()
    return nc


# ------------------------------------------------------- cached PJRT runner

def _make_runner(nc, n_cores):
    """Like bass2jax.run_bass_via_pjrt, but builds the jitted executable once
    so repeated calls don't re-trace/re-compile."""
    import jax
    import concourse.mybir as mb
    from jax.sharding import Mesh, PartitionSpec
    from jax.experimental.shard_map import shard_map
    from concourse.bass2jax import (install_neuronx_cc_hook, partition_id_tensor,
                                    _bass_exec_p)
    install_neuronx_cc_hook()
    partition_name = nc.partition_id_tensor.name if nc.partition_id_tensor else None
    in_names, out_names, out_avals, zero_outs = [], [], [], []
    for alloc in nc.m.functions[0].allocations:
        if not isinstance(alloc, mb.MemoryLocationSet):
            continue
        name = alloc.memorylocations[0].name
        if alloc.kind == "ExternalInput":
            if name != partition_name:
                in_names.append(name)
        elif alloc.kind == "ExternalOutput":
            out_names.append(name)
            shape = tuple(alloc.tensor_shape)
            dtype = mb.dt.np(alloc.dtype)
            out_avals.append(jax.core.ShapedArray(shape, dtype))
            zero_outs.append(np.zeros(shape, dtype))
    n_params = len(in_names)
    n_outs = len(out_avals)
    all_in_names = list(in_names) + list(out_names)
    if partition_name is not None:
        all_in_names.append(partition_name)
    donate = tuple(range(n_params, n_params + n_outs))

    def _body(*args):
        operands = list(args)
        if partition_name is not None:
            operands.append(partition_id_tensor())
        return tuple(_bass_exec_p.bind(
            *operands, out_avals=tuple(out_avals), in_names=tuple(all_in_names),
            out_names=tuple(out_names), lowering_input_output_aliases=(),
            sim_require_finite=True, sim_require_nnan=True, nc=nc))

    devices = jax.devices()[:n_cores]
    mesh = Mesh(np.asarray(devices), ("core",))
    in_specs = (PartitionSpec("core"),) * (n_params + n_outs)
    out_specs = (PartitionSpec("core"),) * n_outs
    sharded = jax.jit(
        shard_map(_body, mesh=mesh, in_specs=in_specs, out_specs=out_specs,
                  check_rep=False),
        donate_argnums=donate, keep_unused=True)

    def call(in_maps):
        concat_in = [
            np.concatenate([np.asarray(in_maps[c][k]) for c in range(n_cores)], 0)
            for k in in_names
        ]
        concat_zeros = [
            np.zeros((n_cores * z.shape[0], *z.shape[1:]), z.dtype)
            for z in zero_outs
        ]
        out_arrs = sharded(*concat_in, *concat_zeros)
        jax.block_until_ready(out_arrs)
        return [
            {k: np.asarray(out_arrs[i]).reshape(n_cores, *out_avals[i].shape)[c]
             for i, k in enumerate(out_names)}
            for c in range(n_cores)
        ]

    return call


# TimelineSim(non-collective 817,932) + rust collective cost model
# (3 x 283,747 for the 25.7MB AllGather); see t_sim.py. The same model
# reproduced the previous kernel's differencing estimate within 1%.
HW_EXEC_NS_ESTIMATE = 1669172

# ----------------------------------------------------------------- entry

_CACHE = {}


def run(cfg, inputs, trace=False):
    ei = np.asarray(inputs["edge_index"])
    key = (cfg.N, cfg.E, cfg.NL, hash(ei.tobytes()))
    if key in _CACHE:
        runner, plan = _CACHE[key]
    else:
        plan = plan_edges(cfg, ei)
        nc = build_program(cfg, plan)
        runner = _make_runner(nc, cfg.NC)
        _CACHE[key] = (runner, plan)
    in_maps = shard_inputs(cfg, inputs, plan)
    results = runner(in_maps)
    lsm = np.concatenate([results[c]["lsm"][:cfg.NV] for c in range(cfg.NC)], 0)
    emb = np.concatenate([results[c]["emb"][:cfg.NV] for c in range(cfg.NC)], 0)
    return (lsm, emb), None


def kernel(**inputs):
    (lsm, emb), _ = run(FULL, inputs)
    return lsm, emb
